# revision 14
# baseline (speedup 1.0000x reference)
"""Trainium2 Bass kernel for the DSIB InfoNCE loss.

Reference computation (B=512, NX=NY=64, HID=256):
    scores[i, j] = MLP(concat(x_j, y_i))       # 3-layer MLP, scalar out
    loss = -(log B + mean(diag(scores)) - mean(logsumexp(scores, axis=1)))

Strategy (data-parallel over the outer y index, 8 cores x 64 rows):
  * Layer 1 is linear in the concatenation, so precompute on device
    A = X @ W1[:64] (shape [512, 256]) and Cb = Y_shard @ W1[64:] + b1
    ([64, 256]); then h1(i, j) = relu(A[j] + Cb[i]).
  * Activations are kept transposed (hid on partitions, pair index on the
    free dim) so layer 2 is a natural PE matmul: for each y row,
    h2.T = relu(W2.T-blocks @ h1.T + b2), 4 accumulating [128,128]x[128,512]
    matmuls in fp16 (fp32 PSUM accumulate).
  * Layer 3 is an M=1 matmul with W3 as the stationary operand, giving the
    full 512-wide score row in PSUM; it is copied into a per-core score
    tile [64, 512].
  * logsumexp (max-subtracted) + masked diagonal extraction run on-device;
    each core returns [64, 2] = (lse_row, diag_row). The host sums the 8
    partial results -- the "all-reduce" of the sharding hint.

fp16 matmul operands keep 11 mantissa bits; validated end-to-end rel err
~3e-4 on the final scalar vs the fp32 reference.
"""

import sys

import numpy as np

_TRN_REPO = "/opt/trn_rl_repo"
if _TRN_REPO not in sys.path:
    sys.path.insert(0, _TRN_REPO)

B = 512
NX = 64
NY = 64
HID = 256
N_CORES = 8
SH = B // N_CORES  # y rows per core

_PROG_CACHE = {}


def _emit(tc, aps, n_rows=SH, do_scatter=True, do_endgame=True, endgame_level=5):
    import concourse.bass as bass  # noqa: F401
    from concourse import mybir

    nc = tc.nc
    f32 = mybir.dt.float32
    f16 = mybir.dt.float16
    AF = mybir.ActivationFunctionType
    ALU = mybir.AluOpType
    AX = mybir.AxisListType

    xt_d = aps["xt"]
    yt_d = aps["yt"]
    w1_d = aps["w1"]
    b1_d = aps["b1"]
    w2_d = aps["w2"]
    b2_d = aps["b2"]
    w3_d = aps["w3"]
    mask_d = aps["mask"]
    out_d = aps["out"]

    with (
        tc.tile_pool(name="const", bufs=1) as cpool,
        tc.tile_pool(name="work", bufs=3) as wpool,
        tc.tile_pool(name="psum", bufs=2, space="PSUM") as ppool,
    ):
        # ---------------- persistent loads ----------------
        xt = cpool.tile([NX, B], f32, name="xt_sb")
        nc.sync.dma_start(xt[:], xt_d[:])
        yt = cpool.tile([NY, SH], f32, name="yt_sb")
        nc.sync.dma_start(yt[:], yt_d[:])
        w1x = cpool.tile([NX, HID], f32, name="w1x_sb")
        nc.sync.dma_start(w1x[:], w1_d[0:NX, :])
        w1y = cpool.tile([NY, HID], f32, name="w1y_sb")
        nc.sync.dma_start(w1y[:], w1_d[NX : NX + NY, :])
        b1c = cpool.tile([128, 2], f32, name="b1_sb")
        nc.sync.dma_start(b1c[:], b1_d.rearrange("(k p) -> p k", p=128))
        b2c = cpool.tile([128, 2], f32, name="b2_sb")
        nc.sync.dma_start(b2c[:], b2_d.rearrange("(k p) -> p k", p=128))
        # w2 sbuf layout: (p, k*HID + m) = W2[k*128 + p, m]
        w2 = cpool.tile([128, 2 * HID], f16, name="w2_sb")
        for k in range(2):
            nc.sync.dma_start(
                w2[:, HID * k : HID * (k + 1)], w2_d[128 * k : 128 * (k + 1), :]
            )
        w3 = cpool.tile([128, 2], f16, name="w3_sb")
        for k in range(2):
            nc.sync.dma_start(
                w3[:, k : k + 1], w3_d[128 * k : 128 * (k + 1), :]
            )
        mask = cpool.tile([SH, B], f32, name="mask_sb")
        nc.sync.dma_start(mask[:], mask_d[:])

        scores = cpool.tile([SH, B], f32, name="scores_sb")
        if n_rows < SH or not do_scatter:
            nc.gpsimd.memset(scores[:], 0.0)

        # ---------------- layer-1 precompute ----------------
        # A.T chunks (fp32 matmul, cast to fp16) and Cb.T chunks (fp32)
        a16 = []
        cb = []
        for m in range(2):
            pa = ppool.tile([128, B], f32, tag="p0", name=f"pa_{m}")
            nc.tensor.matmul(
                pa[:], w1x[:, 128 * m : 128 * m + 128], xt[:], start=True, stop=True
            )
            a = cpool.tile([128, B], f16, name=f"a16_{m}")
            nc.scalar.copy(a[:], pa[:])
            a16.append(a)

            pc = ppool.tile([128, SH], f32, tag="p1", name=f"pc_{m}")
            nc.tensor.matmul(
                pc[:],
                w1y[:, 128 * m : 128 * m + 128],
                yt[:],
                start=True,
                stop=True,
            )
            c = cpool.tile([128, SH], f32, name=f"cb_{m}")
            nc.scalar.activation(c[:], pc[:], AF.Identity, bias=b1c[:, m : m + 1])
            cb.append(c)

        # ---------------- main loop over local y rows ----------------
        for i in range(n_rows):
            h1_0 = wpool.tile([128, B], f16, tag="h1_0", name=f"h1_0_{i}")
            nc.vector.tensor_scalar(
                h1_0[:], a16[0][:], cb[0][:, i : i + 1], 0.0, ALU.add, ALU.max
            )
            h1_1 = wpool.tile([128, B], f16, tag="h1_1", name=f"h1_1_{i}")
            nc.vector.tensor_scalar(
                h1_1[:], a16[1][:], cb[1][:, i : i + 1], 0.0, ALU.add, ALU.max
            )

            p2 = []
            for m in range(2):
                pm = ppool.tile([128, B], f32, tag=f"p{m}", name=f"p2_{m}_{i}")
                nc.tensor.matmul(
                    pm[:],
                    w2[:, 128 * m : 128 * m + 128],
                    h1_0[:],
                    start=True,
                    stop=False,
                )
                nc.tensor.matmul(
                    pm[:],
                    w2[:, HID + 128 * m : HID + 128 * m + 128],
                    h1_1[:],
                    start=False,
                    stop=True,
                )
                p2.append(pm)

            h2_0 = wpool.tile([128, B], f16, tag="h2_0", name=f"h2_0_{i}")
            nc.scalar.activation(h2_0[:], p2[0][:], AF.Relu, bias=b2c[:, 0:1])
            h2_1 = wpool.tile([128, B], f16, tag="h2_1", name=f"h2_1_{i}")
            nc.vector.tensor_scalar(
                h2_1[:], p2[1][:], b2c[:, 1:2], 0.0, ALU.add, ALU.max
            )

            ps = ppool.tile([1, B], f32, tag="ps", name=f"ps_{i}")
            nc.tensor.matmul(ps[:], w3[:, 0:1], h2_0[:], start=True, stop=False)
            nc.tensor.matmul(ps[:], w3[:, 1:2], h2_1[:], start=False, stop=True)

            stage = wpool.tile([1, B], f32, tag="stage", name=f"stage_{i}")
            nc.scalar.copy(stage[:], ps[:])
            if do_scatter:
                nc.sync.dma_start(scores[i : i + 1, :], stage[:])

        if not do_endgame:
            otile = cpool.tile([SH, 2], f32, name="otile")
            nc.scalar.copy(otile[:], scores[:, 0:2])
            nc.sync.dma_start(aps["out"][:], otile[:])
            return

        # ---------------- logsumexp + diag ----------------
        otile = cpool.tile([SH, 2], f32, name="otile")
        nc.gpsimd.memset(otile[:], 0.0)
        negmax = cpool.tile([SH, 1], f32, name="negmax")
        if endgame_level >= 1:
            nc.vector.tensor_reduce(negmax[:], scores[:], AX.X, ALU.max, negate=True)
        else:
            nc.gpsimd.memset(negmax[:], 0.0)
        expt = cpool.tile([SH, B], f32, name="expt")
        sumexp = cpool.tile([SH, 1], f32, name="sumexp")
        if endgame_level >= 2:
            nc.scalar.activation(
                expt[:], scores[:], AF.Exp, bias=negmax[:], accum_out=sumexp[:]
            )
        else:
            nc.gpsimd.memset(sumexp[:], 1.0)
        lse0 = cpool.tile([SH, 1], f32, name="lse0")
        if endgame_level >= 3:
            nc.scalar.activation(lse0[:], sumexp[:], AF.Ln)
        else:
            nc.gpsimd.memset(lse0[:], 0.0)
        if endgame_level >= 4:
            nc.vector.tensor_scalar(
                otile[:, 0:1], lse0[:], negmax[:], None, ALU.subtract
            )
        if endgame_level >= 5:
            mjunk = cpool.tile([SH, B], f32, name="mjunk")
            nc.vector.tensor_mul(mjunk[:], scores[:], mask[:])
            nc.vector.tensor_reduce(otile[:, 1:2], mjunk[:], AX.X, ALU.add)
        nc.sync.dma_start(out_d[:], otile[:])


def _get_program():
    if "nc" in _PROG_CACHE:
        return _PROG_CACHE["nc"]

    import concourse.tile as tile
    from concourse import bacc, mybir

    f32 = mybir.dt.float32
    f16 = mybir.dt.float16

    nc = bacc.Bacc(
        "TRN2", target_bir_lowering=False, debug=False, num_devices=N_CORES
    )
    aps = {
        "xt": nc.dram_tensor("xt", [NX, B], f32, kind="ExternalInput").ap(),
        "yt": nc.dram_tensor("yt", [NY, SH], f32, kind="ExternalInput").ap(),
        "w1": nc.dram_tensor("w1", [NX + NY, HID], f32, kind="ExternalInput").ap(),
        "b1": nc.dram_tensor("b1", [HID], f32, kind="ExternalInput").ap(),
        "w2": nc.dram_tensor("w2", [HID, HID], f16, kind="ExternalInput").ap(),
        "b2": nc.dram_tensor("b2", [HID], f32, kind="ExternalInput").ap(),
        "w3": nc.dram_tensor("w3", [HID, 1], f16, kind="ExternalInput").ap(),
        "mask": nc.dram_tensor("mask", [SH, B], f32, kind="ExternalInput").ap(),
        "out": nc.dram_tensor("out", [SH, 2], f32, kind="ExternalOutput").ap(),
    }

    with tile.TileContext(nc) as tc:
        _emit(tc, aps)
    nc.compile()

    _PROG_CACHE["nc"] = nc
    return nc


def _make_in_maps(dataX, dataY, W1, b1, W2, b2, W3):
    dataX = np.asarray(dataX, np.float32)
    dataY = np.asarray(dataY, np.float32)
    W1 = np.asarray(W1, np.float32)
    b1 = np.asarray(b1, np.float32)
    W2 = np.asarray(W2, np.float32)
    b2 = np.asarray(b2, np.float32)
    W3 = np.asarray(W3, np.float32)

    xt = np.ascontiguousarray(dataX.T)
    w2h = W2.astype(np.float16)
    w3h = W3.astype(np.float16)

    in_maps = []
    for c in range(N_CORES):
        ytc = np.ascontiguousarray(dataY[c * SH : (c + 1) * SH].T)
        maskc = np.zeros((SH, B), np.float32)
        maskc[np.arange(SH), c * SH + np.arange(SH)] = 1.0
        in_maps.append(
            {
                "xt": xt,
                "yt": ytc,
                "w1": W1,
                "b1": b1,
                "w2": w2h,
                "b2": b2,
                "w3": w3h,
                "mask": maskc,
            }
        )
    return in_maps


def _combine(results):
    lse = np.concatenate([np.asarray(r["out"])[:, 0] for r in results])
    diag = np.concatenate([np.asarray(r["out"])[:, 1] for r in results])
    log_b = np.log(np.float64(B))
    mi = log_b + diag.astype(np.float64).mean() - lse.astype(np.float64).mean()
    return np.asarray(-mi, dtype=np.float32)


def _run(inputs):
    from concourse import bass_utils

    nc = _get_program()
    in_maps = _make_in_maps(
        inputs["dataX"],
        inputs["dataY"],
        inputs["W1"],
        inputs["b1"],
        inputs["W2"],
        inputs["b2"],
        inputs["W3"],
    )
    res = bass_utils.run_bass_kernel_spmd(
        nc, in_maps, core_ids=list(range(N_CORES)), trace=False
    )
    return _combine(res.results), res


class _Executor:
    """Reusable sharded executable over the 8 cores, for timing loops.

    Replicates bass2jax.run_bass_via_pjrt's multi-core path but keeps the
    jitted callable and device-resident inputs so repeated calls measure
    dispatch + NEFF execution only (no fresh trace/compile, no host->device
    input transfer).
    """

    def __init__(self, nc, in_maps):
        import jax
        import numpy as np
        from jax.sharding import Mesh, NamedSharding, PartitionSpec
        from jax.experimental.shard_map import shard_map

        from concourse import bass2jax, mybir

        bass2jax.install_neuronx_cc_hook()

        partition_name = (
            nc.partition_id_tensor.name if nc.partition_id_tensor else None
        )
        in_names, out_names, out_avals, zero_outs = [], [], [], []
        for alloc in nc.m.functions[0].allocations:
            if not isinstance(alloc, mybir.MemoryLocationSet):
                continue
            name = alloc.memorylocations[0].name
            if alloc.kind == "ExternalInput":
                if name != partition_name:
                    in_names.append(name)
            elif alloc.kind == "ExternalOutput":
                out_names.append(name)
                shape = tuple(alloc.tensor_shape)
                dtype = mybir.dt.np(alloc.dtype)
                out_avals.append(jax.core.ShapedArray(shape, dtype))
                zero_outs.append(np.zeros(shape, dtype))
        n_params = len(in_names)
        n_outs = len(out_avals)
        all_in_names = list(in_names) + list(out_names)
        if partition_name is not None:
            all_in_names.append(partition_name)
        donate = tuple(range(n_params, n_params + n_outs))

        def _body(*args):
            operands = list(args)
            if partition_name is not None:
                operands.append(bass2jax.partition_id_tensor())
            outs = bass2jax._bass_exec_p.bind(
                *operands,
                out_avals=tuple(out_avals),
                in_names=tuple(all_in_names),
                out_names=tuple(out_names),
                lowering_input_output_aliases=(),
                sim_require_finite=True,
                sim_require_nnan=True,
                nc=nc,
            )
            return tuple(outs)

        devices = jax.devices()[:N_CORES]
        mesh = Mesh(np.asarray(devices), ("core",))
        in_specs = (PartitionSpec("core"),) * (n_params + n_outs)
        out_specs = (PartitionSpec("core"),) * len(out_names)
        self._fn = jax.jit(
            shard_map(
                _body,
                mesh=mesh,
                in_specs=in_specs,
                out_specs=out_specs,
                check_rep=False,
            ),
            donate_argnums=donate,
            keep_unused=True,
        )
        per_core = [
            [np.asarray(m[name]) for name in in_names] for m in in_maps
        ]
        sharding = NamedSharding(mesh, PartitionSpec("core"))
        self._dev_in = [
            jax.device_put(
                np.concatenate([per_core[c][i] for c in range(N_CORES)], axis=0),
                sharding,
            )
            for i in range(n_params)
        ]
        self._zero_shapes = [
            ((N_CORES * z.shape[0],) + z.shape[1:], z.dtype) for z in zero_outs
        ]
        self._out_names = out_names
        self._out_avals = out_avals
        self._jax = jax

    def __call__(self):
        zeros = [np.zeros(s, d) for s, d in self._zero_shapes]
        outs = self._fn(*self._dev_in, *zeros)
        self._jax.block_until_ready(outs)
        return outs

    def results(self, outs):
        res = []
        for c in range(N_CORES):
            res.append(
                {
                    name: np.asarray(outs[i]).reshape(
                        N_CORES, *self._out_avals[i].shape
                    )[c]
                    for i, name in enumerate(self._out_names)
                }
            )
        return res


def kernel(**inputs):
    return _run(inputs)[0]
